# revision 1
# baseline (speedup 1.0000x reference)
"""Trainium2 Bass kernel for a pre-norm transformer block (E=512, H=2048, NH=8, N=4096).

Sharding: sequence-parallel over 8 NeuronCores. Each core computes the full K/V
projection (needs all 4096 tokens) but only its own 512-token slice of queries,
attention output, MLP and residuals. No collectives; host concatenates slices.

On-chip layout is feature-major (features on SBUF partitions, tokens on the free
dim) so every matmul contracts along partitions with no transposes:
  - weights are host-transposed to [in_features, out_features]
  - scores are computed transposed ([k_tokens, q_tokens]) so the softmax sum is
    a matmul reduction, and exp(scores) feeds the PV matmul directly
  - the softmax denominator rides along as a 65th "ones" column of V
Matmul operands are bf16 (FWL weight loads overlap with matmuls; fp32r pays a
serial ~220ns LDWEIGHTS per matmul). The residual stream stays fp32: x_slice,
x1, LayerNorm statistics, softmax denominators and the final output are fp32,
so only the attention/MLP branch outputs carry bf16 rounding (~1e-3 relative).
Set PRECISION = "f32r" for the full-precision fallback.
"""
import sys

sys.path.insert(0, "/opt/trn_rl_repo")
sys.path.insert(0, "/opt/pypackages")

import numpy as np

E, H, NH, HD = 512, 2048, 8, 64
T, NCORES = 4096, 8
TC = T // NCORES          # tokens per core
P = 128
ET = E // P               # 4  feature tiles of E
HT = H // P               # 16 feature tiles of H
KT = T // P               # 32 key-token tiles
NCH = T // TC             # 8  token chunks for the K/V pass
EPS = 1e-5
PRECISION = "bf16"

_BUILT = None


def _build():
    import concourse.bacc as bacc
    import concourse.mybir as mybir
    import concourse.tile as tile


    dt = mybir.dt
    AF = mybir.ActivationFunctionType
    OP = mybir.AluOpType
    F32 = dt.float32
    WT = dt.bfloat16 if PRECISION == "bf16" else dt.float32r

    nc = bacc.Bacc("TRN2", target_bir_lowering=False, debug=False, num_devices=NCORES)

    d_xT = nc.dram_tensor("xT", [E, T], WT, kind="ExternalInput").ap()
    d_xsT = nc.dram_tensor("xsT", [E, TC], F32, kind="ExternalInput").ap()
    d_wqkvT = nc.dram_tensor("wqkvT", [E, 3 * E], WT, kind="ExternalInput").ap()
    d_bqkv = nc.dram_tensor("bqkv", [3 * E], F32, kind="ExternalInput").ap()
    d_wprojT = nc.dram_tensor("wprojT", [E, E], WT, kind="ExternalInput").ap()
    d_bproj = nc.dram_tensor("bproj", [E], F32, kind="ExternalInput").ap()
    d_wfc1T = nc.dram_tensor("wfc1T", [E, H], WT, kind="ExternalInput").ap()
    d_bfc1 = nc.dram_tensor("bfc1", [H], F32, kind="ExternalInput").ap()
    d_wfc2T = nc.dram_tensor("wfc2T", [H, H], WT, kind="ExternalInput").ap()
    d_bfc2 = nc.dram_tensor("bfc2", [H], F32, kind="ExternalInput").ap()
    d_wfc3T = nc.dram_tensor("wfc3T", [H, E], WT, kind="ExternalInput").ap()
    d_bfc3 = nc.dram_tensor("bfc3", [E], F32, kind="ExternalInput").ap()
    d_lng = nc.dram_tensor("lng", [E], F32, kind="ExternalInput").ap()
    d_lnb = nc.dram_tensor("lnb", [E], F32, kind="ExternalInput").ap()
    d_outT = nc.dram_tensor("outT", [E, TC], F32, kind="ExternalOutput").ap()

    with tile.TileContext(nc) as tc:
        _emit(nc, tc, tile, mybir, locals())

    nc.compile()
    return nc


def _emit(nc, tc, tile, mybir, d):
    dt = mybir.dt
    AF = mybir.ActivationFunctionType
    OP = mybir.AluOpType
    F32 = dt.float32
    WT = dt.bfloat16 if PRECISION == "bf16" else dt.float32r

    def pool(**kw):
        p = tc.tile_pool(**kw)
        return p.__enter__(), p

    def close(*ps):
        for p in ps:
            p.__exit__(None, None, None)

    # ---- long-lived pools ----
    consts, _c0 = pool(name="consts", bufs=1, side="left")
    stats, _c1 = pool(name="stats", bufs=3, side="left")
    bcast, _c2 = pool(name="bcast", bufs=3, side="left")
    scratch, _c3 = pool(name="scratch", bufs=6, side="left")
    STAT = [pool(name="ps_stat", bufs=2, space="PSUM")]
    drp, _cd = pool(name="drscratch", bufs=4, space="DRAM")

    # ---- constants ----
    onesf = consts.tile([P, 1], F32)
    nc.vector.memset(onesf[:], 1.0)
    ones_w = consts.tile([P, 1], WT)
    nc.vector.tensor_copy(ones_w[:], onesf[:])
    eps_t = consts.tile([1, 1], F32)
    nc.vector.memset(eps_t[:], EPS)
    eps_p = consts.tile([P, 1], F32)
    nc.vector.memset(eps_p[:], EPS)

    def ld_vec(dram, n, name):  # [n] f32 -> [P, n//P] per-partition layout
        t = consts.tile([P, n // P], F32, name=name)
        nc.sync.dma_start(t[:], dram.rearrange("(m p) -> p m", p=P))
        return t

    g_sb = ld_vec(d["d_lng"], E, "g_sb")
    b_sb = ld_vec(d["d_lnb"], E, "b_sb")
    bq_sb = ld_vec(d["d_bqkv"][0:E], E, "bq_sb")
    bk_sb = ld_vec(d["d_bqkv"][E:2 * E], E, "bk_sb")
    bv_sb = ld_vec(d["d_bqkv"][2 * E:3 * E], E, "bv_sb")
    bproj_sb = ld_vec(d["d_bproj"], E, "bproj_sb")
    bfc1_sb = ld_vec(d["d_bfc1"], H, "bfc1_sb")
    bfc2_sb = ld_vec(d["d_bfc2"], H, "bfc2_sb")
    bfc3_sb = ld_vec(d["d_bfc3"], E, "bfc3_sb")

    def recip_bcast(dst_bb, src_1w, w, rsqrt=False):
        """dst_bb[P, w] = broadcast(1/src) or broadcast(1/sqrt(src + eps)).

        A single-partition DVE reciprocal costs ~6.5ns/element, so spread the
        w values over 128 partitions via a DRAM bounce, invert there, then
        broadcast back with a stride-0 DMA read.
        """
        dr1 = drp.tile([w], F32, tag="dr1", name="dr1")
        nc.sync.dma_start(dr1[None, :], src_1w)
        pk = scratch.tile([P, w // P], F32, tag="rpk", name="rpk")
        nc.sync.dma_start(pk[:], dr1.rearrange("(p f) -> p f", p=P))
        if rsqrt:
            nc.scalar.activation(pk[:], pk[:], AF.Sqrt, bias=eps_p[:])
        nc.vector.reciprocal(pk[:], pk[:])
        dr2 = drp.tile([w], F32, tag="dr2", name="dr2")
        nc.sync.dma_start(dr2.rearrange("(p f) -> p f", p=P), pk[:])
        src_b = dr2[None, :].to_broadcast((P, w))
        if dst_bb.dtype == F32:
            nc.sync.dma_start(dst_bb, src_b)
        else:
            nc.gpsimd.dma_start(dst_bb, src_b)

    def bcast_dram(dst_bb, src_1w, w):
        """dst_bb[P, w] = broadcast(src[1, w]) via DRAM stride-0 read."""
        dr = drp.tile([w], F32, tag="drb", name="drb")
        nc.sync.dma_start(dr[None, :], src_1w)
        src_b = dr[None, :].to_broadcast((P, w))
        if dst_bb.dtype == F32:
            nc.sync.dma_start(dst_bb, src_b)
        else:
            nc.gpsimd.dma_start(dst_bb, src_b)

    def ln_stats(src, w, src_dt, fast=False):
        """First half of LayerNorm: returns (mu_b, rs_b) broadcast tiles [P, w].

        Stat matmuls always run in WT (f32 sources are cast first) so they
        take 1 cycle/row on the PE. The rstd reciprocal runs packed across
        partitions via a DRAM bounce.
        """
        if src_dt == WT:
            stat_src = [src[:, e, :] for e in range(ET)]
        else:
            stat_src = []
            for e in range(ET):
                xw = scratch.tile([P, w], WT, tag="ln_xw", name="ln_xw")
                nc.vector.tensor_copy(xw[:], src[:, e, :])
                stat_src.append(xw[:])
        x2s = [scratch.tile([P, w], WT, tag="ln_x2", name="ln_x2")
               for _ in range(ET)]
        for e in range(ET):
            nc.vector.tensor_mul(x2s[e][:], stat_src[e], stat_src[e])
        mu_ps = STAT[0][0].tile([1, w], F32, tag="mu")
        sq_ps = STAT[0][0].tile([1, w], F32, tag="sq")
        for e in range(ET):
            nc.tensor.matmul(mu_ps[:], ones_w[:], stat_src[e],
                             start=(e == 0), stop=(e == ET - 1))
        for e in range(ET):
            nc.tensor.matmul(sq_ps[:], ones_w[:], x2s[e][:],
                             start=(e == 0), stop=(e == ET - 1))
        return ln_chain(mu_ps, sq_ps, w, fast=fast)

    def ln_chain(mu_ps, sq_ps, w, fast=False):
        mu = stats.tile([1, w], F32, tag="mu_sb", name="mu_sb")
        ms = stats.tile([1, w], F32, tag="ms_sb", name="ms_sb")
        if fast:
            # latency-critical path (ACT is idle here): no DRAM bounce
            nc.scalar.mul(mu[:], mu_ps[:], 1.0 / E)
            nc.scalar.mul(ms[:], sq_ps[:], 1.0 / E)
        else:
            nc.vector.tensor_scalar_mul(mu[:], mu_ps[:], 1.0 / E)
            nc.vector.tensor_scalar_mul(ms[:], sq_ps[:], 1.0 / E)
        var = stats.tile([1, w], F32, tag="var", name="var")
        nc.vector.tensor_mul(var[:], mu[:], mu[:])
        nc.vector.tensor_sub(var[:], ms[:], var[:])
        mu_b = bcast.tile([P, w], WT, tag="mu_b", name="mu_b")
        rs_b = bcast.tile([P, w], WT, tag="rs_b", name="rs_b")
        if fast:
            rstd = consts.tile([1, w], F32, tag="rstd_f", name="rstd_f")
            nc.scalar.activation(rstd[:], var[:], AF.Sqrt, bias=eps_t[:])
            nc.vector.reciprocal(rstd[:], rstd[:])
            mu_w = consts.tile([1, w], WT, tag="mu_w", name="mu_w")
            rs_w = consts.tile([1, w], WT, tag="rs_w", name="rs_w")
            nc.vector.tensor_copy(mu_w[:], mu[:])
            nc.vector.tensor_copy(rs_w[:], rstd[:])
            nc.gpsimd.partition_broadcast(mu_b[:], mu_w[:])
            nc.gpsimd.partition_broadcast(rs_b[:], rs_w[:])
        else:
            bcast_dram(mu_b[:], mu[:], w)
            recip_bcast(rs_b[:], var[:], w, rsqrt=True)
        return mu_b, rs_b

    def ln_apply(dst, src, mu_b, rs_b, w):
        for e in range(ET):
            nc.vector.tensor_sub(dst[:, e, :], src[:, e, :], mu_b[:])
            nc.vector.scalar_tensor_tensor(dst[:, e, :], dst[:, e, :],
                                           g_sb[:, e:e + 1], rs_b[:],
                                           op0=OP.mult, op1=OP.mult)
            nc.vector.tensor_scalar_add(dst[:, e, :], dst[:, e, :],
                                        scalar1=b_sb[:, e:e + 1])

    def layernorm(dst, src, w, src_dt, fast=False):
        mu_b, rs_b = ln_stats(src, w, src_dt, fast=fast)
        ln_apply(dst, src, mu_b, rs_b, w)

    # ---- K/V/Q tensors live until end of attention ----
    big, h_big = pool(name="big", bufs=1, side="right")
    KTs = big.tile([P, ET, T], WT)              # K^T, feature-major
    V65 = big.tile([P, KT, NH, HD + 1], WT)     # V token-major + ones col
    QTs = big.tile([P, ET, TC], WT)

    # ones column of V65 (denominator rides the PV matmul)
    nc.vector.tensor_copy(
        V65[:, :, :, HD:HD + 1],
        onesf[:, :, None, None].to_broadcast((P, KT, NH, 1)))

    # ====== phase 1: stream x in 512-token chunks, LN, K/V proj (sw-pipelined:
    # chunk n+1's LN statistics are emitted before chunk n's projections so the
    # PE priority order hides the LN latency) ======
    ps_mm4, h_ps_mm4 = pool(name="ps_mm4", bufs=4, space="PSUM")
    CW = 512
    NCH1 = T // CW
    wqp, h_wqp = pool(name="wq", bufs=1, side="right")
    wq = wqp.tile([P, ET, E], WT)
    wkvp, h_wkvp = pool(name="wkv", bufs=1, side="right")
    xcp, h_xcp = pool(name="xc", bufs=4, side="right")
    wkv = wkvp.tile([P, ET, 2 * E], WT)

    def warmup(n, rhs):
        wps = ps_mm4.tile([1, rhs.shape[-1]], F32, tag="mm", name="wps")
        for i in range(n):
            nc.tensor.matmul(wps[:], ones_w[:], rhs,
                             start=(i == 0), stop=(i == n - 1),
                             skip_group_check=True)

    def kv_project(xc, ch):
        for m in range(ET):
            kps = ps_mm4.tile([P, CW], F32, tag="mm", name="kps")
            for e in range(ET):
                nc.tensor.matmul(kps[:], wkv[:, e, m * P:(m + 1) * P],
                                 xc[:, e, :], start=(e == 0), stop=(e == ET - 1))
            nc.scalar.activation(KTs[:, m, ch * CW:(ch + 1) * CW], kps[:],
                                 AF.Identity, bias=bk_sb[:, m:m + 1])
        for t4 in range(CW // P):
            vps = ps_mm4.tile([P, E], F32, tag="mm", name="vps")
            for e in range(ET):
                nc.tensor.matmul(vps[:], xc[:, e, t4 * P:(t4 + 1) * P],
                                 wkv[:, e, E:2 * E],
                                 start=(e == 0), stop=(e == ET - 1))
            kt = ch * (CW // P) + t4
            nc.scalar.activation(
                V65[:, kt, :, 0:HD],
                vps[:].rearrange("p (h d) -> p h d", h=NH), AF.Copy)

    # Q projection resources (emitted mid-pipeline so it overlaps the K/V pass)
    persistA, h_persistA = pool(name="persistA", bufs=1, side="left")
    xs_sb = persistA.tile([P, ET, TC], F32)
    UTs = persistA.tile([P, ET, TC], WT)        # first LN(x_slice), then attn out

    def emit_phase2():
        nc.sync.dma_start(xs_sb[:], d["d_xsT"].rearrange("(m p) t -> p m t", p=P))
        hq = UTs                                 # reuse UTs space for LN(x_slice)
        layernorm(hq, xs_sb, TC, F32, fast=True)
        for m in range(ET):
            qps = ps_mm4.tile([P, TC], F32, tag="mm", name="qps")
            for e in range(ET):
                nc.tensor.matmul(qps[:], wq[:, e, m * P:(m + 1) * P], hq[:, e, :],
                                 start=(e == 0), stop=(e == ET - 1))
            nc.scalar.activation(QTs[:, m, :], qps[:], AF.Identity,
                                 bias=bq_sb[:, m:m + 1])

    pending = []
    for ch in range(NCH1):
        xc = xcp.tile([P, ET, CW], WT, tag="xc", name="xc")
        nc.sync.dma_start(
            xc[:],
            d["d_xT"][:, ch * CW:(ch + 1) * CW].rearrange("(m p) t -> p m t", p=P))
        if ch == 0:
            warmup(24, xc[:, 0, :])          # bridge the LN latency at t=0
            # weight loads queued after the first x chunk so chunk-0 LN starts
            nc.sync.dma_start(
                wkv[:], d["d_wqkvT"][:, E:3 * E].rearrange("(m p) o -> p m o", p=P))
            nc.sync.dma_start(
                wq[:], d["d_wqkvT"][:, 0:E].rearrange("(m p) o -> p m o", p=P))
        mu_b, rs_b = ln_stats(xc, CW, WT)
        ln_apply(xc, xc, mu_b, rs_b, CW)
        pending.append((xc, ch))
        if len(pending) == 2:
            kv_project(*pending.pop(0))
    while pending:
        kv_project(*pending.pop(0))
    emit_phase2()
    warmup(44, KTs[:, 0, 0:TC])              # bridge Q-chain -> attention start
    close(h_xcp, h_wkvp, h_wqp)
    close(h_ps_mm4)
    close(STAT[0][1])

    # ====== phase 3: attention (2 heads in flight, exp over kt-pairs) ======
    ps_sc, h_ps_sc = pool(name="ps_sc", bufs=3, space="PSUM")
    ps_pv, h_ps_pv = pool(name="ps_pv", bufs=1, space="PSUM")
    ptp, h_ptp = pool(name="ptile", bufs=4, side="right")
    stp, h_stp = pool(name="stage", bufs=2, side="right")
    scale = float(HD) ** -0.5
    for mp in range(ET):
        heads = [2 * mp, 2 * mp + 1]
        pvs = [ps_pv.tile([HD + 1, TC], F32, tag=f"pv{j}", name="pv")
               for j in range(2)]
        for ktp in range(KT // 2):
            k0 = 2 * ktp
            pts = []
            for j, h in enumerate(heads):
                lo = (h % 2) * HD
                m = h // 2
                sc2 = ps_sc.tile([P, 2 * TC], F32, tag="sc2", name="sc2")
                nc.tensor.matmul(sc2[:, 0:TC],
                                 KTs[lo:lo + HD, m, k0 * P:(k0 + 1) * P],
                                 QTs[lo:lo + HD, m, :], skip_group_check=True)
                nc.tensor.matmul(sc2[:, TC:2 * TC],
                                 KTs[lo:lo + HD, m, (k0 + 1) * P:(k0 + 2) * P],
                                 QTs[lo:lo + HD, m, :], skip_group_check=True)
                pt2 = ptp.tile([P, 2 * TC], WT, tag="pt2", name="pt2")
                nc.scalar.activation(pt2[:], sc2[:], AF.Exp, scale=scale)
                pts.append(pt2)
            for j, h in enumerate(heads):
                nc.tensor.matmul(pvs[j][:], V65[:, k0, h, :], pts[j][:, 0:TC],
                                 start=(k0 == 0), stop=False,
                                 skip_group_check=True)
                nc.tensor.matmul(pvs[j][:], V65[:, k0 + 1, h, :],
                                 pts[j][:, TC:2 * TC],
                                 start=False, stop=(k0 + 1 == KT - 1),
                                 skip_group_check=True)
        for j, h in enumerate(heads):
            lo = (h % 2) * HD
            m = h // 2
            pv = pvs[j]
            stg = stp.tile([HD, TC], WT, tag="stg", name="stg")
            nc.vector.tensor_copy(stg[:], pv[0:HD, :])
            stg_s = stp.tile([HD + 1, TC], F32, tag="stg_s", name="stg_s")
            nc.vector.tensor_copy(stg_s[HD:HD + 1, :], pv[HD:HD + 1, :])
            nc.sync.dma_start(UTs[lo:lo + HD, m, :], stg[:])
            rb = bcast.tile([P, TC], F32, tag="rb", name="rb")
            recip_bcast(rb[:], stg_s[HD:HD + 1, :], TC)
            nc.vector.tensor_mul(UTs[lo:lo + HD, m, :], UTs[lo:lo + HD, m, :],
                                 rb[lo:lo + HD, :])
            nc.vector.tensor_scalar_add(UTs[lo:lo + HD, m, :],
                                        UTs[lo:lo + HD, m, :],
                                        scalar1=bv_sb[lo:lo + HD, m:m + 1])
    close(h_stp, h_ptp, h_ps_pv, h_ps_sc)
    close(h_big)                     # K/V/Q dead after attention

    # ============ phase 4: output proj + residual + LN2 ============
    STAT[0] = pool(name="ps_stat2", bufs=2, space="PSUM")
    ps_mm, h_ps_mm = pool(name="ps_mm", bufs=4, space="PSUM")
    persistB, h_persistB = pool(name="persistB", bufs=1, side="left")
    x1_sb = persistB.tile([P, ET, TC], F32)
    h2_sb = persistB.tile([P, ET, TC], WT)
    outsb = persistB.tile([P, ET, TC], F32)
    wpp, h_wpp = pool(name="wproj", bufs=1, side="left")
    wproj = wpp.tile([P, ET, E], WT)
    nc.sync.dma_start(wproj[:], d["d_wprojT"].rearrange("(m p) o -> p m o", p=P))

    def warmup2(n, rhs):
        wps = ps_mm.tile([1, rhs.shape[-1]], F32, tag="mm", name="wps")
        for i in range(n):
            nc.tensor.matmul(wps[:], ones_w[:], rhs,
                             start=(i == 0), stop=(i == n - 1),
                             skip_group_check=True)

    warmup2(56, wproj[:, 0, :])              # bridge attention tail -> proj
    mu2_ps = STAT[0][0].tile([1, TC], F32, tag="mu")
    sq2_ps = STAT[0][0].tile([1, TC], F32, tag="sq")
    for m in range(ET):
        pps = ps_mm.tile([P, TC], F32, tag="mm", name="pps")
        for e in range(ET):
            nc.tensor.matmul(pps[:], wproj[:, e, m * P:(m + 1) * P],
                             UTs[:, e, :], start=(e == 0), stop=(e == ET - 1))
        # x1 = (proj + bias) + x_slice
        nc.vector.scalar_tensor_tensor(
            x1_sb[:, m, :], pps[:], bproj_sb[:, m:m + 1], xs_sb[:, m, :],
            op0=OP.add, op1=OP.add)
        # LN2 statistics accumulate as each x1 block lands
        xw = scratch.tile([P, TC], WT, tag="ln_xw", name="ln_xw")
        nc.vector.tensor_copy(xw[:], x1_sb[:, m, :])
        x2 = scratch.tile([P, TC], WT, tag="ln_x2", name="ln_x2")
        nc.vector.tensor_mul(x2[:], xw[:], xw[:])
        nc.tensor.matmul(mu2_ps[:], ones_w[:], xw[:],
                         start=(m == 0), stop=(m == ET - 1), skip_group_check=True)
        nc.tensor.matmul(sq2_ps[:], ones_w[:], x2[:],
                         start=(m == 0), stop=(m == ET - 1), skip_group_check=True)
    close(h_wpp)
    mu_b2, rs_b2 = ln_chain(mu2_ps, sq2_ps, TC, fast=True)
    ln_apply(h2_sb, x1_sb, mu_b2, rs_b2, TC)

    # ============ phase 5: MLP ============
    mlp, h_mlp = pool(name="mlp", bufs=1, side="left")
    m1_sb = mlp.tile([P, HT, TC], WT)
    m2_sb = mlp.tile([P, HT, TC], WT)
    w1p, h_w1p = pool(name="wfc1", bufs=1, side="left")
    wfc1 = w1p.tile([P, ET, H], WT)
    nc.sync.dma_start(wfc1[:], d["d_wfc1T"].rearrange("(m p) o -> p m o", p=P))
    w3p, h_w3p = pool(name="wfc3", bufs=1, side="left")
    wfc3 = w3p.tile([P, HT, E], WT)
    nc.sync.dma_start(wfc3[:], d["d_wfc3T"].rearrange("(m p) o -> p m o", p=P))
    w2p, h_w2p = pool(name="wfc2c", bufs=1, side="left")
    wcs = []
    for e in range(HT):
        wc = w2p.tile([P, H], WT, tag=f"wc{e}", name="wc")
        nc.sync.dma_start(wc[:], d["d_wfc2T"][e * P:(e + 1) * P, :])
        wcs.append(wc)
    warmup2(36, wfc1[:, 0, 0:TC])            # bridge LN2 chain -> fc1
    for m in range(HT):
        ps1 = ps_mm.tile([P, TC], F32, tag="mm", name="ps1")
        for e in range(ET):
            nc.tensor.matmul(ps1[:], wfc1[:, e, m * P:(m + 1) * P],
                             h2_sb[:, e, :], start=(e == 0), stop=(e == ET - 1))
        nc.scalar.activation(m1_sb[:, m, :], ps1[:], AF.Relu,
                             bias=bfc1_sb[:, m:m + 1])
    close(h_ps_mm, STAT[0][1])

    # fc2: all 16 weight chunks resident -> one dense 256-matmul run
    ps8p, h_ps8p = pool(name="ps8", bufs=6, space="PSUM")
    for m in range(HT):
        psm = ps8p.tile([P, TC], F32, tag="mm8", name="psm")
        for e in range(HT):
            nc.tensor.matmul(psm[:], wcs[e][:, m * P:(m + 1) * P],
                             m1_sb[:, e, :],
                             start=(e == 0), stop=(e == HT - 1),
                             skip_group_check=True)
        nc.scalar.activation(m2_sb[:, m, :], psm[:], AF.Relu,
                             bias=bfc2_sb[:, m:m + 1])
    close(h_ps8p, h_w2p)

    ps_f3, h_ps_f3 = pool(name="ps_f3", bufs=2, space="PSUM")
    for m in range(ET):
        ps3 = ps_f3.tile([P, TC], F32, tag="f3", name="ps3")
        for e in range(HT):
            nc.tensor.matmul(ps3[:], wfc3[:, e, m * P:(m + 1) * P],
                             m2_sb[:, e, :], start=(e == 0), stop=(e == HT - 1))
        nc.vector.scalar_tensor_tensor(
            outsb[:, m, :], ps3[:], bfc3_sb[:, m:m + 1], x1_sb[:, m, :],
            op0=OP.add, op1=OP.add)
        nc.sync.dma_start(d["d_outT"][m * P:(m + 1) * P, :], outsb[:, m, :])
    close(h_ps_f3, h_w3p, h_w1p, h_mlp, h_persistB, h_persistA)
    close(_cd)
    close(_c3, _c2, _c1, _c0)


def _get_nc():
    global _BUILT
    if _BUILT is None:
        _BUILT = _build()
    return _BUILT


def run(inputs, trace=False):
    from concourse.bass_utils import run_bass_kernel_spmd

    nc = _get_nc()
    import ml_dtypes
    wt_np = ml_dtypes.bfloat16 if PRECISION == "bf16" else np.float32
    x = np.asarray(inputs["x"], np.float32)[0]          # [T, E]
    ct = lambda a: np.ascontiguousarray(np.asarray(a, np.float32).T)
    ctw = lambda a: ct(a).astype(wt_np)
    common = {
        "xT": ctw(x),
        "wqkvT": ctw(inputs["qkv_w"]),
        "bqkv": np.asarray(inputs["qkv_b"], np.float32),
        "wprojT": ctw(inputs["proj_w"]),
        "bproj": np.asarray(inputs["proj_b"], np.float32),
        "wfc1T": ctw(inputs["fc1_w"]),
        "bfc1": np.asarray(inputs["fc1_b"], np.float32),
        "wfc2T": ctw(inputs["fc2_w"]),
        "bfc2": np.asarray(inputs["fc2_b"], np.float32),
        "wfc3T": ctw(inputs["fc3_w"]),
        "bfc3": np.asarray(inputs["fc3_b"], np.float32),
        "lng": np.asarray(inputs["ln_g"], np.float32),
        "lnb": np.asarray(inputs["ln_b"], np.float32),
    }
    in_maps = [
        {**common, "xsT": ct(x[c * TC:(c + 1) * TC, :])} for c in range(NCORES)
    ]
    res = run_bass_kernel_spmd(nc, in_maps, core_ids=list(range(NCORES)),
                               trace=trace)
    out = np.empty((1, T, E), np.float32)
    for c in range(NCORES):
        out[0, c * TC:(c + 1) * TC, :] = res.results[c]["outT"].T
    return out, res


def kernel(**inputs) -> np.ndarray:
    out, _ = run(inputs, trace=False)
    return out



# revision 14
# speedup vs baseline: 1.1749x; 1.1749x over previous
"""Trainium2 Bass kernel for a pre-norm transformer block (E=512, H=2048, NH=8, N=4096).

Sharding: sequence-parallel over 8 NeuronCores. Each core computes the full K/V
projection (needs all 4096 tokens) but only its own 512-token slice of queries,
attention output, MLP and residuals. No collectives; host concatenates slices.

Per-core inputs are ROTATED so chunk 0 is always the core's own token slice
(softmax and PV are key-order invariant, so K/V living in rotated order is
harmless). This lets the SPMD program compute Q during chunk 0 with no
core-dependent control flow.

Phase 1 (LN1 + QKV projection) design:
  - x arrives host-converted to fp8(e4m3); LN gamma/beta are folded into the
    consumer weights host-side (W' = W*g, b' = b + W@b_ln), so the on-chip LN
    is just z = (x - mu) * rstd.
  - LN statistics use a ones[128, 2, 128] fp8 DoubleRow stationary so the
    matmul output [128, CW] is the per-token sum REPLICATED across all 128
    partitions: the broadcast is free, no DRAM bounce.
  - rstd = exp(-0.5 * ln(var + eps)) on the ACT engine (fp32 tables, ~1e-6).
  - QKV projections run fp8 DoubleRow (2 virtual-K=256 matmuls instead of 4),
    weights pre-scaled x16 host-side, descaled in the PSUM->SBUF write op.
Attention: scores bf16 (row-tiled 2 heads concurrently), exp on ACT with a
constant bias -EXPB (softmax-invariant, keeps exp in fp8/bf16 range), PV in
fp8 DoubleRow with the softmax denominator riding as a 65th column of V.
MLP: bf16 throughout (accuracy: the MLP branch is O(1) of the output).
"""
import sys

sys.path.insert(0, "/opt/trn_rl_repo")
sys.path.insert(0, "/opt/pypackages")

import numpy as np

E, H, NH, HD = 512, 2048, 8, 64
T, NCORES = 4096, 8
TC = T // NCORES          # tokens per core
P = 128
ET = E // P               # 4  feature tiles of E
HT = H // P               # 16 feature tiles of H
KT = T // P               # 32 key-token tiles
EPS = 1e-5
WS = 16.0                 # fp8 weight pre-scale
CW = 512                  # phase-1 chunk width (tokens)
NCH = T // CW

_BUILT = None


def _build():
    import concourse.bacc as bacc
    import concourse.mybir as mybir
    import concourse.tile as tile

    nc = bacc.Bacc("TRN2", target_bir_lowering=False, debug=False, num_devices=NCORES)
    dt = mybir.dt
    F32, F8 = dt.float32, dt.float8e4
    BF = dt.bfloat16

    d = {}
    d["d_xT8"] = nc.dram_tensor("xT8", [E, T], F8, kind="ExternalInput").ap()
    d["d_xsT"] = nc.dram_tensor("xsT", [E, TC], F32, kind="ExternalInput").ap()
    d["d_wqkvT8"] = nc.dram_tensor("wqkvT8", [E, 3 * E], F8, kind="ExternalInput").ap()
    d["d_bqkv"] = nc.dram_tensor("bqkv", [3 * E], F32, kind="ExternalInput").ap()
    d["d_wprojT"] = nc.dram_tensor("wprojT", [E, E], BF, kind="ExternalInput").ap()
    d["d_bproj"] = nc.dram_tensor("bproj", [E], F32, kind="ExternalInput").ap()
    d["d_wfc1T"] = nc.dram_tensor("wfc1T", [E, H], BF, kind="ExternalInput").ap()
    d["d_bfc1"] = nc.dram_tensor("bfc1", [H], F32, kind="ExternalInput").ap()
    d["d_wfc2T"] = nc.dram_tensor("wfc2T", [H, H], BF, kind="ExternalInput").ap()
    d["d_bfc2"] = nc.dram_tensor("bfc2", [H], F32, kind="ExternalInput").ap()
    d["d_wfc3T"] = nc.dram_tensor("wfc3T", [H, E], BF, kind="ExternalInput").ap()
    d["d_bfc3"] = nc.dram_tensor("bfc3", [E], F32, kind="ExternalInput").ap()
    d["d_outT"] = nc.dram_tensor("outT", [E, TC], F32, kind="ExternalOutput").ap()

    with tile.TileContext(nc) as tc:
        _emit(nc, tc, tile, mybir, d)

    nc.compile()
    return nc


def _emit(nc, tc, tile, mybir, d):
    dt = mybir.dt
    AF = mybir.ActivationFunctionType
    OP = mybir.AluOpType
    PM = mybir.MatmulPerfMode
    F32, F8, BF = dt.float32, dt.float8e4, dt.bfloat16

    def pool(**kw):
        p = tc.tile_pool(**kw)
        return p.__enter__(), p

    def close(*ps):
        for p in ps:
            p.__exit__(None, None, None)

    # ---- long-lived pools ----
    consts, _c0 = pool(name="consts", bufs=1, side="left")
    lnp, _c1 = pool(name="lnp", bufs=2, side="left")
    STAT = [pool(name="ps_stat", bufs=2, space="PSUM")]

    # ---- constants ----
    ones_w = consts.tile([P, 1], BF)
    nc.vector.memset(ones_w[:], 1.0)
    ones8p = consts.tile([P, 2, P], F8)        # DoubleRow stats stationary
    nc.vector.memset(ones8p[:], 1.0)
    ones_bf = consts.tile([P, P], BF)          # bf16 stats stationary (LN2)
    nc.vector.memset(ones_bf[:], 1.0)
    onesf = consts.tile([P, 1], F32)
    nc.vector.memset(onesf[:], 1.0)
    eps_p = consts.tile([P, 1], F32)
    nc.vector.memset(eps_p[:], EPS)

    def ld_vec(dram, n, name):  # [n] f32 -> [P, n//P] per-partition layout
        t = consts.tile([P, n // P], F32, name=name)
        nc.sync.dma_start(t[:], dram.rearrange("(m p) -> p m", p=P))
        return t

    bq_sb = ld_vec(d["d_bqkv"][0:E], E, "bq_sb")
    bk_sb = ld_vec(d["d_bqkv"][E:2 * E], E, "bk_sb")
    bv_sb = ld_vec(d["d_bqkv"][2 * E:3 * E], E, "bv_sb")
    bproj_sb = ld_vec(d["d_bproj"], E, "bproj_sb")
    bfc1_sb = ld_vec(d["d_bfc1"], H, "bfc1_sb")
    bfc2_sb = ld_vec(d["d_bfc2"], H, "bfc2_sb")
    bfc3_sb = ld_vec(d["d_bfc3"], E, "bfc3_sb")

    def ln_chain(mu_ps, sq_ps, w, tag):
        """Broadcast-stats LN chain: mu_ps/sq_ps are [P, w] PSUM tiles holding
        per-token sum(x) / sum(x^2) replicated over partitions. Returns
        (mu_b, rs_b) bf16 [P, w]."""
        mu_b = lnp.tile([P, w], BF, tag=f"mu{tag}", name="mu_b")
        nc.scalar.activation(mu_b[:], mu_ps[:], AF.Identity, scale=1.0 / E)
        mu2 = lnp.tile([P, w], F32, tag=f"m2{tag}", name="mu2")
        nc.gpsimd.tensor_mul(mu2[:], mu_b[:], mu_b[:])
        var = lnp.tile([P, w], F32, tag=f"va{tag}", name="var")
        nc.vector.scalar_tensor_tensor(var[:], sq_ps[:], 1.0 / E, mu2[:],
                                       op0=OP.mult, op1=OP.subtract)
        lnv = lnp.tile([P, w], F32, tag=f"lv{tag}", name="lnv")
        nc.scalar.activation(lnv[:], var[:], AF.Ln, bias=eps_p[:])
        rs_b = lnp.tile([P, w], BF, tag=f"rs{tag}", name="rs_b")
        nc.scalar.activation(rs_b[:], lnv[:], AF.Exp, scale=-0.5)
        return mu_b, rs_b

    # ---- persistent attention tensors ----
    PADV = 80  # pair stride must be 16-aligned for DoubleRow stationary APs
    big, h_big = pool(name="big", bufs=1, side="right")
    KTs = big.tile([P, ET, T], BF)              # K^T feature-major
    V65 = big.tile([P, KT // 2, NH, 2, PADV], F8)  # V token-major DR pairs + ones col
    QTs = big.tile([P, ET, TC], BF)

    # ones column of V65 (softmax denominator rides the PV matmul)
    nc.vector.memset(V65[:, :, :, :, HD:HD + 1], 1.0)

    persistA, h_persistA = pool(name="persistA", bufs=1, side="left")
    xs_sb = persistA.tile([P, ET, TC], F32)
    UTs = persistA.tile([P, ET, TC], BF)        # attention out (pre-proj)
    nc.sync.dma_start(xs_sb[:], d["d_xsT"].rearrange("(m p) t -> p m t", p=P))

    # ====== phase 1: stream x chunks, LN1 + QKV projection (fp8 DoubleRow) ======
    ps_mm4, h_ps_mm4 = pool(name="ps_mm4", bufs=4, space="PSUM")
    wq8p, h_wq8p = pool(name="wq8", bufs=1, side="right")
    wqkv8 = wq8p.tile([P, ET, 3 * E], F8)
    xcp, h_xcp = pool(name="xc", bufs=3, side="right")
    xnp, h_xnp = pool(name="xn", bufs=3, side="right")
    sqp, h_sqp = pool(name="sq", bufs=2, side="right")

    def warmup(n, rhs, wps_pool):
        wps = wps_pool.tile([1, rhs.shape[-1]], F32, tag="mm", name="wps")
        for i in range(n):
            nc.tensor.matmul(wps[:], ones_w[:], rhs,
                             start=(i == 0), stop=(i == n - 1),
                             skip_group_check=True)

    def ln_stats(xc8, w):
        xsq = sqp.tile([P, ET, w], F8, tag="xsq", name="xsq")
        for e in range(ET):
            nc.gpsimd.tensor_mul(xsq[:, e, :], xc8[:, e, :], xc8[:, e, :])
        mu_ps = STAT[0][0].tile([P, w], F32, tag="mu")
        sq_ps = STAT[0][0].tile([P, w], F32, tag="sq")
        for h in range(2):
            nc.tensor.matmul(mu_ps[:], ones8p[:], xc8[:, 2 * h:2 * h + 2, :],
                             start=(h == 0), stop=(h == 1), perf_mode=PM.DoubleRow)
        for h in range(2):
            nc.tensor.matmul(sq_ps[:], ones8p[:], xsq[:, 2 * h:2 * h + 2, :],
                             start=(h == 0), stop=(h == 1), perf_mode=PM.DoubleRow)
        return ln_chain(mu_ps, sq_ps, w, tag="1")

    def ln_apply8(xn8, xc8, mu_b, rs_b, w):
        for e in range(ET):
            tmp = lnp.tile([P, w], BF, tag=f"ap{e&1}", name="tmp")
            nc.gpsimd.tensor_sub(tmp[:], xc8[:, e, :], mu_b[:])
            nc.vector.tensor_mul(xn8[:, e, :], tmp[:], rs_b[:])

    def qkv_project(xn8, ch):
        with_q = (ch == 0)
        # K projection: feature-major out [128 kfeat, CW]
        for m in range(ET):
            kps = ps_mm4.tile([P, CW], F32, tag="mm", name="kps")
            for h in range(2):
                nc.tensor.matmul(
                    kps[:], wqkv8[:, 2 * h:2 * h + 2, E + m * P:E + (m + 1) * P],
                    xn8[:, 2 * h:2 * h + 2, :],
                    start=(h == 0), stop=(h == 1), perf_mode=PM.DoubleRow)
            nc.vector.tensor_scalar(KTs[:, m, ch * CW:(ch + 1) * CW], kps[:],
                                    1.0 / WS, bk_sb[:, m:m + 1],
                                    op0=OP.mult, op1=OP.add)
        # V projection: token-major out [128 tok, E]
        for t4 in range(CW // P):
            vps = ps_mm4.tile([P, E], F32, tag="mm", name="vps")
            for h in range(2):
                nc.tensor.matmul(
                    vps[:], xn8[:, 2 * h:2 * h + 2, t4 * P:(t4 + 1) * P],
                    wqkv8[:, 2 * h:2 * h + 2, 2 * E:3 * E],
                    start=(h == 0), stop=(h == 1), perf_mode=PM.DoubleRow)
            kt = ch * (CW // P) + t4
            nc.scalar.activation(
                V65[:, kt // 2, :, kt % 2, 0:HD],
                vps[:].rearrange("p (h d) -> p h d", h=NH),
                AF.Identity, scale=1.0 / WS)
        if with_q:
            for m in range(ET):
                qps = ps_mm4.tile([P, TC], F32, tag="mm", name="qps")
                for h in range(2):
                    nc.tensor.matmul(
                        qps[:], wqkv8[:, 2 * h:2 * h + 2, m * P:(m + 1) * P],
                        xn8[:, 2 * h:2 * h + 2, :],
                        start=(h == 0), stop=(h == 1), perf_mode=PM.DoubleRow)
                nc.scalar.activation(QTs[:, m, :], qps[:], AF.Identity,
                                     bias=bq_sb[:, m:m + 1], scale=1.0 / WS)

    pending = []
    for ch in range(NCH):
        xc8 = xcp.tile([P, ET, CW], F8, tag="xc", name="xc")
        nc.sync.dma_start(
            xc8[:],
            d["d_xT8"][:, ch * CW:(ch + 1) * CW].rearrange("(m p) t -> p m t", p=P))
        if ch == 0:
            warmup(10, ones_bf[:, 0:P], ps_mm4)
            nc.sync.dma_start(
                wqkv8[:], d["d_wqkvT8"].rearrange("(m p) o -> p m o", p=P))
        mu_b, rs_b = ln_stats(xc8, CW)
        xn8 = xnp.tile([P, ET, CW], F8, tag="xn", name="xn")
        ln_apply8(xn8, xc8, mu_b, rs_b, CW)
        pending.append((xn8, ch))
        if len(pending) == 2:
            qkv_project(*pending.pop(0))
    while pending:
        qkv_project(*pending.pop(0))
    close(h_sqp, h_xnp, h_xcp, h_wq8p)
    close(h_ps_mm4)
    close(STAT[0][1])

    # ====== phase 3: attention ======
    # scores bf16 row-tiled (2 heads concurrent); exp with constant bias -EXPB
    # (softmax-invariant); P/V fp8; PV DoubleRow over kt pairs.
    EXPB = 2.0
    nexpb_p = consts.tile([P, 1], F32)
    nc.vector.memset(nexpb_p[:], -EXPB)
    drp, _cd = pool(name="drscratch", bufs=2, space="DRAM")
    scratch, _c3 = pool(name="scratch", bufs=4, side="left")
    bcast, _c2 = pool(name="bcast", bufs=3, side="left")
    ps_sc, h_ps_sc = pool(name="ps_sc", bufs=3, space="PSUM")
    ps_pv, h_ps_pv = pool(name="ps_pv", bufs=1, space="PSUM")
    ptp, h_ptp = pool(name="ptile", bufs=4, side="right")
    stp, h_stp = pool(name="stage", bufs=2, side="right")
    scale = float(HD) ** -0.5

    def recip_bcast(dst_bb, src_1w, w):
        """dst_bb[P, w] = broadcast(1/src[1, w]) via a DRAM bounce: spread the
        w values over 128 partitions, invert, broadcast back stride-0."""
        dr1 = drp.tile([w], F32, tag="dr1", name="dr1")
        nc.sync.dma_start(dr1[None, :], src_1w)
        pk = scratch.tile([P, w // P], F32, tag="rpk", name="rpk")
        nc.sync.dma_start(pk[:], dr1.rearrange("(p f) -> p f", p=P))
        nc.vector.reciprocal(pk[:], pk[:])
        dr2 = drp.tile([w], F32, tag="dr2", name="dr2")
        nc.sync.dma_start(dr2.rearrange("(p f) -> p f", p=P), pk[:])
        src_b = dr2[None, :].to_broadcast((P, w))
        nc.sync.dma_start(dst_bb, src_b)

    for mp in range(ET):
        heads = [2 * mp, 2 * mp + 1]
        pvs = [ps_pv.tile([HD + 1, TC], F32, tag=f"pv{j}", name="pv")
               for j in range(2)]
        for ktp in range(KT // 2):
            k0 = 2 * ktp
            pts = []
            for j, h in enumerate(heads):
                lo = (h % 2) * HD
                m = h // 2
                sc2 = ps_sc.tile([P, 2 * TC], F32, tag="sc2", name="sc2")
                nc.tensor.matmul(sc2[:, 0:TC],
                                 KTs[lo:lo + HD, m, k0 * P:(k0 + 1) * P],
                                 QTs[lo:lo + HD, m, :], skip_group_check=True)
                nc.tensor.matmul(sc2[:, TC:2 * TC],
                                 KTs[lo:lo + HD, m, (k0 + 1) * P:(k0 + 2) * P],
                                 QTs[lo:lo + HD, m, :], skip_group_check=True)
                pt2 = ptp.tile([P, 2, TC], F8, tag="pt2", name="pt2")
                nc.scalar.activation(pt2[:], sc2[:].rearrange("p (k t) -> p k t", k=2),
                                     AF.Exp, scale=scale, bias=nexpb_p[:])
                pts.append(pt2)
            for j, h in enumerate(heads):
                nc.tensor.matmul(pvs[j][:], V65[:, ktp, h, :, 0:HD + 1], pts[j][:],
                                 start=(ktp == 0), stop=(ktp == KT // 2 - 1),
                                 perf_mode=PM.DoubleRow,
                                 skip_group_check=True)
        for j, h in enumerate(heads):
            lo = (h % 2) * HD
            m = h // 2
            pv = pvs[j]
            stg = stp.tile([HD, TC], BF, tag="stg", name="stg")
            nc.vector.tensor_copy(stg[:], pv[0:HD, :])
            stg_s = stp.tile([HD + 1, TC], F32, tag="stg_s", name="stg_s")
            nc.vector.tensor_copy(stg_s[HD:HD + 1, :], pv[HD:HD + 1, :])
            nc.sync.dma_start(UTs[lo:lo + HD, m, :], stg[:])
            rb = bcast.tile([P, TC], F32, tag="rb", name="rb")
            recip_bcast(rb[:], stg_s[HD:HD + 1, :], TC)
            nc.vector.tensor_mul(UTs[lo:lo + HD, m, :], UTs[lo:lo + HD, m, :],
                                 rb[lo:lo + HD, :])
            nc.vector.tensor_scalar_add(UTs[lo:lo + HD, m, :],
                                        UTs[lo:lo + HD, m, :],
                                        scalar1=bv_sb[lo:lo + HD, m:m + 1])
    close(h_stp, h_ptp, h_ps_pv, h_ps_sc)
    close(h_big)                     # K/V/Q dead after attention

    # ============ phase 4: output proj + residual + LN2 ============
    STAT[0] = pool(name="ps_stat2", bufs=2, space="PSUM")
    ps_mm, h_ps_mm = pool(name="ps_mm", bufs=4, space="PSUM")
    persistB, h_persistB = pool(name="persistB", bufs=1, side="left")
    x1_sb = persistB.tile([P, ET, TC], F32)
    h2_sb = persistB.tile([P, ET, TC], BF)
    outsb = persistB.tile([P, ET, TC], F32)
    wpp, h_wpp = pool(name="wproj", bufs=1, side="left")
    wproj = wpp.tile([P, ET, E], BF)
    nc.sync.dma_start(wproj[:], d["d_wprojT"].rearrange("(m p) o -> p m o", p=P))

    def warmup2(n, rhs):
        warmup(n, rhs, ps_mm)

    warmup2(56, wproj[:, 0, :])              # bridge attention tail -> proj
    mu2_ps = STAT[0][0].tile([P, TC], F32, tag="mu")
    sq2_ps = STAT[0][0].tile([P, TC], F32, tag="sq")
    for m in range(ET):
        pps = ps_mm.tile([P, TC], F32, tag="mm", name="pps")
        for e in range(ET):
            nc.tensor.matmul(pps[:], wproj[:, e, m * P:(m + 1) * P],
                             UTs[:, e, :], start=(e == 0), stop=(e == ET - 1))
        # x1 = (proj + bias) + x_slice
        nc.vector.scalar_tensor_tensor(
            x1_sb[:, m, :], pps[:], bproj_sb[:, m:m + 1], xs_sb[:, m, :],
            op0=OP.add, op1=OP.add)
        # LN2 statistics accumulate as each x1 block lands
        xw = scratch.tile([P, TC], BF, tag="ln_xw", name="ln_xw")
        nc.vector.tensor_copy(xw[:], x1_sb[:, m, :])
        x2 = scratch.tile([P, TC], BF, tag="ln_x2", name="ln_x2")
        nc.gpsimd.tensor_mul(x2[:], xw[:], xw[:])
        nc.tensor.matmul(mu2_ps[:], ones_bf[:], xw[:],
                         start=(m == 0), stop=(m == ET - 1), skip_group_check=True)
        nc.tensor.matmul(sq2_ps[:], ones_bf[:], x2[:],
                         start=(m == 0), stop=(m == ET - 1), skip_group_check=True)
    close(h_wpp)
    mu_b2, rs_b2 = ln_chain(mu2_ps, sq2_ps, TC, tag="1")
    for e in range(ET):
        tmp = lnp.tile([P, TC], BF, tag=f"ap{e & 1}", name="tmp2")
        nc.gpsimd.tensor_sub(tmp[:], x1_sb[:, e, :], mu_b2[:])
        nc.vector.tensor_mul(h2_sb[:, e, :], tmp[:], rs_b2[:])

    # ============ phase 5: MLP ============
    mlp, h_mlp = pool(name="mlp", bufs=1, side="left")
    m1_sb = mlp.tile([P, HT, TC], BF)
    m2_sb = mlp.tile([P, HT, TC], BF)
    w1p, h_w1p = pool(name="wfc1", bufs=1, side="left")
    wfc1 = w1p.tile([P, ET, H], BF)
    nc.sync.dma_start(wfc1[:], d["d_wfc1T"].rearrange("(m p) o -> p m o", p=P))
    w3p, h_w3p = pool(name="wfc3", bufs=1, side="left")
    wfc3 = w3p.tile([P, HT, E], BF)
    nc.sync.dma_start(wfc3[:], d["d_wfc3T"].rearrange("(m p) o -> p m o", p=P))
    w2p, h_w2p = pool(name="wfc2c", bufs=1, side="left")
    wcs = []
    for e in range(HT):
        wc = w2p.tile([P, H], BF, tag=f"wc{e}", name="wc")
        nc.sync.dma_start(wc[:], d["d_wfc2T"][e * P:(e + 1) * P, :])
        wcs.append(wc)
    warmup2(36, wfc1[:, 0, 0:TC])            # bridge LN2 chain -> fc1
    for m in range(HT):
        ps1 = ps_mm.tile([P, TC], F32, tag="mm", name="ps1")
        for e in range(ET):
            nc.tensor.matmul(ps1[:], wfc1[:, e, m * P:(m + 1) * P],
                             h2_sb[:, e, :], start=(e == 0), stop=(e == ET - 1))
        nc.scalar.activation(m1_sb[:, m, :], ps1[:], AF.Relu,
                             bias=bfc1_sb[:, m:m + 1])
    close(h_ps_mm, STAT[0][1])

    # fc2: all 16 weight chunks resident -> one dense 256-matmul run
    ps8p, h_ps8p = pool(name="ps8", bufs=6, space="PSUM")
    for m in range(HT):
        psm = ps8p.tile([P, TC], F32, tag="mm8", name="psm")
        for e in range(HT):
            nc.tensor.matmul(psm[:], wcs[e][:, m * P:(m + 1) * P],
                             m1_sb[:, e, :],
                             start=(e == 0), stop=(e == HT - 1),
                             skip_group_check=True)
        nc.scalar.activation(m2_sb[:, m, :], psm[:], AF.Relu,
                             bias=bfc2_sb[:, m:m + 1])
    close(h_ps8p, h_w2p)

    ps_f3, h_ps_f3 = pool(name="ps_f3", bufs=2, space="PSUM")
    for m in range(ET):
        ps3 = ps_f3.tile([P, TC], F32, tag="f3", name="ps3")
        for e in range(HT):
            nc.tensor.matmul(ps3[:], wfc3[:, e, m * P:(m + 1) * P],
                             m2_sb[:, e, :], start=(e == 0), stop=(e == HT - 1))
        nc.vector.scalar_tensor_tensor(
            outsb[:, m, :], ps3[:], bfc3_sb[:, m:m + 1], x1_sb[:, m, :],
            op0=OP.add, op1=OP.add)
        nc.sync.dma_start(d["d_outT"][m * P:(m + 1) * P, :], outsb[:, m, :])
    close(h_ps_f3, h_w3p, h_w1p, h_mlp, h_persistB)
    close(_cd)
    close(_c2, _c3, h_persistA, _c1, _c0)


def _get_nc():
    global _BUILT
    if _BUILT is None:
        _BUILT = _build()
    return _BUILT


def run(inputs, trace=False):
    from concourse.bass_utils import run_bass_kernel_spmd
    import ml_dtypes

    nc = _get_nc()
    bf = ml_dtypes.bfloat16
    f8 = ml_dtypes.float8_e4m3fn
    x = np.asarray(inputs["x"], np.float32)[0]          # [T, E]
    g = np.asarray(inputs["ln_g"], np.float32)
    b = np.asarray(inputs["ln_b"], np.float32)

    def fold(w, bias):  # fold LN gamma/beta into consumer weight/bias
        w = np.asarray(w, np.float32)
        return w * g[None, :], np.asarray(bias, np.float32) + w @ b

    qkv_w, bqkv = fold(inputs["qkv_w"], inputs["qkv_b"])
    fc1_w, bfc1 = fold(inputs["fc1_w"], inputs["fc1_b"])

    ct = lambda a: np.ascontiguousarray(np.asarray(a, np.float32).T)
    ctb = lambda a: ct(a).astype(bf)
    to8 = lambda a: np.clip(a, -240.0, 240.0).astype(f8)
    common = {
        "wqkvT8": to8(ct(qkv_w) * WS),
        "bqkv": bqkv,
        "wprojT": ctb(inputs["proj_w"]),
        "bproj": np.asarray(inputs["proj_b"], np.float32),
        "wfc1T": ct(fc1_w).astype(bf),
        "bfc1": bfc1,
        "wfc2T": ctb(inputs["fc2_w"]),
        "bfc2": np.asarray(inputs["fc2_b"], np.float32),
        "wfc3T": ctb(inputs["fc3_w"]),
        "bfc3": np.asarray(inputs["fc3_b"], np.float32),
    }
    in_maps = []
    for c in range(NCORES):
        xrot = np.concatenate([x[c * TC:], x[:c * TC]], axis=0)   # own slice first
        in_maps.append({
            **common,
            "xT8": to8(ct(xrot)),
            "xsT": ct(x[c * TC:(c + 1) * TC, :]),
        })
    res = run_bass_kernel_spmd(nc, in_maps, core_ids=list(range(NCORES)),
                               trace=trace)
    out = np.empty((1, T, E), np.float32)
    for c in range(NCORES):
        out[0, c * TC:(c + 1) * TC, :] = res.results[c]["outT"].T
    return out, res


def kernel(**inputs) -> np.ndarray:
    out, _ = run(inputs, trace=False)
    return out
